# revision 9
# baseline (speedup 1.0000x reference)
"""Trainium2 Bass kernel for CausalGatedSSMBlock.

Sharding: batch(4) x time(2) across 8 cores. Core c handles batch c//2,
token half c%2 (1024 tokens). Each core computes the full block for its
chunk; the only cross-core dependency is the recurrence carry at the
half boundary, exchanged as a [2048] f32 AllReduce within each pair.

On-chip layout is channel-major ([ch, t], "transposed") so the three
inner matmuls and the DVE tensor_tensor_scan all operate natively; the
flips between token-major and channel-major use DMA transpose (bf16).
"""

import os
import sys

sys.path.insert(0, "/opt/trn_rl_repo")
os.environ.setdefault("MYCRO_LOCAL_CACHE", "1")

import numpy as np
import ml_dtypes

import concourse.bass as bass
import concourse.mybir as mybir
import concourse.tile as tile_mod
from concourse.tile import TileContext
from concourse.bass import ts
from concourse.bass_utils import run_bass_kernel_spmd

bf16 = mybir.dt.bfloat16
f32 = mybir.dt.float32
AO = mybir.AluOpType
AF = mybir.ActivationFunctionType

B, S, D, INNER = 4, 2048, 1024, 2048
T = 1024          # tokens per core
NTOK = T // 128   # 8 token tiles
KD = D // 128     # 8 k-tiles of d_model
MI = INNER // 128  # 16 channel tiles of inner dim
EPS = 1e-5

# ---------------------------------------------------------------------------
# This walrus build rejects >1 sem wait on several instruction types
# ("Too many sync wait commands" in setupSyncWait). Keep at most one wait
# per instruction: Tile's kernel-tail drain is rebuilt from single-wait
# drains, and a post-pass hoists overflow waits onto nop carriers that run
# just before the owning instruction on the same engine.
_MAXW = 1
_ctr = [0]


def _patched_drain_and_barrier(self, tick_clock, wait_clock):
    drain_inst = self.nc.sync.drain()
    ins = drain_inst.ins
    wait_clock.add_sem_waits(ins, tile_mod.ScopedClock({None: tick_clock.global_clock}))
    waits = list(ins.sync_info.on_wait)
    if len(waits) > _MAXW:
        ins.sync_info.on_wait = waits[:_MAXW]
        for i in range(_MAXW, len(waits), _MAXW):
            extra = self.nc.sync.drain()
            wait_clock.add_sem_waits(
                extra.ins, tile_mod.ScopedClock({None: tick_clock.global_clock})
            )
            extra.ins.sync_info.on_wait = waits[i : i + _MAXW]
    self.nc.all_engine_barrier()
    popped = self.nc._tile_sem_poison_stack.pop()
    assert popped is self._sem_poison
    self.nc.clear_and_free_semaphores(list(self.sems.allocated().values()))
    self.nc.all_engine_barrier()


TileContext._drain_and_barrier = _patched_drain_and_barrier


def _split_waits(nc, maxw=_MAXW):
    for f in nc.m.functions:
        for bb in f.blocks:
            new = []
            for inst in bb.instructions:
                si = inst.sync_info
                if si is not None:
                    waits = list(si.on_wait)
                    if len(waits) > maxw:
                        keep = waits[-maxw:]
                        extra = waits[:-maxw]
                        for i in range(0, len(extra), maxw):
                            _ctr[0] += 1
                            new.append(
                                mybir.InstNoOp(
                                    name=f"wsplit_{_ctr[0]}",
                                    sync_info=mybir.SyncInfo(
                                        on_wait=extra[i : i + maxw], on_update=[]
                                    ),
                                    bass_nofuse=True,
                                    engine=inst.engine,
                                )
                            )
                        si.on_wait = keep
                new.append(inst)
            bb.instructions = new


# ---------------------------------------------------------------------------


def build_nc():
    nc = bass.Bass()

    xb = nc.declare_dram_parameter("xb", [T, D], f32, isOutput=False)
    halo = nc.declare_dram_parameter("halo", [128, MI, 2], bf16, isOutput=False)
    w_in = nc.declare_dram_parameter("w_in", [D, 2 * INNER], bf16, isOutput=False)
    wa = nc.declare_dram_parameter("wa", [INNER, INNER], bf16, isOutput=False)
    wb = nc.declare_dram_parameter("wb", [INNER, INNER], bf16, isOutput=False)
    wc = nc.declare_dram_parameter("wc", [INNER, INNER], bf16, isOutput=False)
    wo = nc.declare_dram_parameter("wo", [INNER, D], bf16, isOutput=False)
    b_in_t = nc.declare_dram_parameter("b_in_t", [128, 2 * MI], f32, isOutput=False)
    ba_t = nc.declare_dram_parameter("ba_t", [128, MI], f32, isOutput=False)
    bb_t = nc.declare_dram_parameter("bb_t", [128, MI], f32, isOutput=False)
    bc_t = nc.declare_dram_parameter("bc_t", [128, MI], f32, isOutput=False)
    bo_t = nc.declare_dram_parameter("bo_t", [128, KD], f32, isOutput=False)
    cw_t = nc.declare_dram_parameter("cw_t", [128, MI, 3], f32, isOutput=False)
    cb_t = nc.declare_dram_parameter("cb_t", [128, MI], f32, isOutput=False)
    sel_p = nc.declare_dram_parameter("sel", [1, 1], f32, isOutput=False)
    invsel_p = nc.declare_dram_parameter("invsel", [1, 1], f32, isOutput=False)
    out = nc.declare_dram_parameter("out", [T, D], f32, isOutput=True)

    from contextlib import ExitStack

    with TileContext(nc) as tc, ExitStack() as big:
        consts = big.enter_context(tc.tile_pool(name="consts", bufs=1))
        wstream = big.enter_context(tc.tile_pool(name="wstream", bufs=17))
        psum = big.enter_context(tc.tile_pool(name="psum", bufs=8, space="PSUM"))
        _pc = [0]
        def ps_tile():
            _pc[0] += 1
            return psum.tile([128, 512], f32, tag="ps", name=f"ps_{_pc[0]}")
        dram = big.enter_context(tc.tile_pool(name="dram", bufs=1, space="DRAM"))

        # ---- constants --------------------------------------------------
        binp_sb = consts.tile([128, 2 * MI], f32)
        nc.sync.dma_start(out=binp_sb[:], in_=b_in_t[:])
        ba_sb = consts.tile([128, MI], f32)
        nc.sync.dma_start(out=ba_sb[:], in_=ba_t[:])
        bb_sb = consts.tile([128, MI], f32)
        nc.sync.dma_start(out=bb_sb[:], in_=bb_t[:])
        bc_sb = consts.tile([128, MI], f32)
        nc.sync.dma_start(out=bc_sb[:], in_=bc_t[:])
        bo_sb = consts.tile([128, KD], f32)
        nc.sync.dma_start(out=bo_sb[:], in_=bo_t[:])
        cw_sb = consts.tile([128, MI, 3], f32)
        nc.sync.dma_start(out=cw_sb[:], in_=cw_t[:])
        cb_sb = consts.tile([128, MI], f32)
        nc.sync.dma_start(out=cb_sb[:], in_=cb_t[:])
        sel_sb = consts.tile([128, 1], f32)
        nc.sync.dma_start(out=sel_sb[:], in_=sel_p[:].to_broadcast((128, 1)))
        invsel_sb = consts.tile([128, 1], f32)
        nc.sync.dma_start(out=invsel_sb[:], in_=invsel_p[:].to_broadcast((128, 1)))
        eps_sb = consts.tile([128, 1], f32)
        nc.vector.memset(eps_sb[:], EPS)

        carry_loc = consts.tile([128, MI], f32)
        carry_eff = consts.tile([128, MI], f32)
        cc_in = dram.tile([128, MI], f32)
        cc_out = dram.tile([128, MI], f32)

        # ---- weight streaming: uniform [128, 2048] bf16 blocks ----------
        # W_in: 8 k-blocks x 2 m-halves; Wa/Wb/Wc: 16 k-blocks;
        # Wo: 16 k-blocks of [128, 1024] (same tag, smaller tile).
        win_blk = {}
        for k in range(KD):
            for mh in range(2):
                t_ = wstream.tile([128, INNER], bf16, tag="wblk", name=f"win_{k}_{mh}")
                nc.sync.dma_start(
                    out=t_[:], in_=w_in[ts(k, 128), ts(mh, INNER)]
                )
                win_blk[(k, mh)] = t_

        # ay outlives the mid-section (y tiles feed the Wo matmul), so its
        # pool must be entered before u8/xnT per LIFO pool ordering.
        ay = big.enter_context(tc.tile_pool(name="ay", bufs=MI + 1))
        mid = ExitStack()  # u8 + a/b/c-phase transients; closed before Wo
        u8 = mid.enter_context(tc.tile_pool(name="u8", bufs=1)).tile(
            [128, MI, T], bf16
        )

        # ---- stage A: layernorm + transpose -----------------------------
        stAB = ExitStack()  # pools freed after stage B
        xnT_pool = stAB.enter_context(tc.tile_pool(name="xnT", bufs=1))
        xnT = xnT_pool.tile([128, KD, T], bf16)
        with ExitStack() as stA:
            xln = stA.enter_context(tc.tile_pool(name="xln", bufs=3))
            xnp = stA.enter_context(tc.tile_pool(name="xnp", bufs=3))
            stat = stA.enter_context(tc.tile_pool(name="stat", bufs=6))
            for j in range(NTOK):
                xt = xln.tile([128, D], f32)
                nc.sync.dma_start(out=xt[:], in_=xb[ts(j, 128), :])
                stats = stat.tile([128, 2, 6], f32)
                xr = xt[:].rearrange("p (s q) -> p s q", s=2)
                for s_ in range(2):
                    nc.vector.bn_stats(out=stats[:, s_, :], in_=xr[:, s_, :])
                mv = stat.tile([128, 2], f32)
                nc.vector.bn_aggr(out=mv[:], in_=stats[:])
                rstd = stat.tile([128, 1], f32)
                nc.scalar.activation(
                    out=rstd[:], in_=mv[:, 1:2], func=AF.Sqrt,
                    bias=eps_sb[:], scale=1.0,
                )
                nc.vector.reciprocal(out=rstd[:], in_=rstd[:])
                xn = xnp.tile([128, D], bf16)
                nc.vector.tensor_scalar(
                    out=xn[:], in0=xt[:], scalar1=mv[:, 0:1], scalar2=rstd[:],
                    op0=AO.subtract, op1=AO.mult,
                )
                nc.sync.dma_start_transpose(out=xnT[:, :, ts(j, 128)], in_=xn[:])

        # ---- stage B: W_in matmul + conv + gated silu -> u --------------
        with ExitStack() as stB:
            projp = stB.enter_context(tc.tile_pool(name="projp", bufs=3))
            sgp = stB.enter_context(tc.tile_pool(name="sgp", bufs=3))
            cvt = stB.enter_context(tc.tile_pool(name="cvt", bufs=4))
            for m in range(MI):
                proj = projp.tile([128, 2 + T], bf16, tag="proj")
                nc.sync.dma_start(out=proj[:, 0:2], in_=halo[:, m, :])
                for n in range(2):
                    ps = ps_tile()
                    for k in range(KD):
                        nc.tensor.matmul(
                            ps[:], win_blk[(k, 0)][:, ts(m, 128)],
                            xnT[:, k, ts(n, 512)],
                            start=(k == 0), stop=(k == KD - 1),
                        )
                    nc.scalar.activation(
                        out=proj[:, 2 + n * 512 : 2 + (n + 1) * 512], in_=ps[:],
                        func=AF.Identity, bias=binp_sb[:, m : m + 1], scale=1.0,
                    )
                sg = sgp.tile([128, T], bf16, tag="sg")
                for n in range(2):
                    ps = ps_tile()
                    for k in range(KD):
                        nc.tensor.matmul(
                            ps[:], win_blk[(k, 1)][:, ts(m, 128)],
                            xnT[:, k, ts(n, 512)],
                            start=(k == 0), stop=(k == KD - 1),
                        )
                    nc.scalar.activation(
                        out=sg[:, ts(n, 512)], in_=ps[:],
                        func=AF.Sigmoid, bias=binp_sb[:, MI + m : MI + m + 1],
                        scale=1.0,
                    )
                # causal depthwise conv (3 taps) + bias + silu
                c1 = cvt.tile([128, T], f32, tag="cv")
                nc.vector.tensor_scalar(
                    out=c1[:], in0=proj[:, 0:T], scalar1=cw_sb[:, m, 0:1],
                    scalar2=None, op0=AO.mult,
                )
                c2 = cvt.tile([128, T], f32, tag="cv")
                nc.vector.scalar_tensor_tensor(
                    out=c2[:], in0=proj[:, 1 : 1 + T], scalar=cw_sb[:, m, 1:2],
                    in1=c1[:], op0=AO.mult, op1=AO.add,
                )
                c3 = cvt.tile([128, T], f32, tag="cv")
                nc.vector.scalar_tensor_tensor(
                    out=c3[:], in0=proj[:, 2 : 2 + T], scalar=cw_sb[:, m, 2:3],
                    in1=c2[:], op0=AO.mult, op1=AO.add,
                )
                sil = cvt.tile([128, T], bf16, tag="sil")
                nc.scalar.activation(
                    out=sil[:], in_=c3[:], func=AF.Silu,
                    bias=cb_sb[:, m : m + 1], scale=1.0,
                )
                nc.vector.tensor_tensor(
                    out=u8[:, m, :], in0=sil[:], in1=sg[:], op=AO.mult
                )
        stAB.close()  # free xnT before the a/b/c phases

        # ---- stage a/b/c matmuls + scans --------------------------------
        bu = mid.enter_context(tc.tile_pool(name="bu", bufs=MI))
        cp = mid.enter_context(tc.tile_pool(name="cp", bufs=3))
        stp = mid.enter_context(tc.tile_pool(name="stp", bufs=2))
        ytmp = mid.enter_context(tc.tile_pool(name="ytmp", bufs=2))

        def stream_w(param, nk, width, pfx):
            blks = []
            for k in range(nk):
                t_ = wstream.tile([128, width], bf16, tag="wblk", name=f"{pfx}_{k}")
                nc.sync.dma_start(out=t_[:], in_=param[ts(k, 128), :])
                blks.append(t_)
            return blks

        def mm_phase(blks, bias_sb, func, dest_of):
            # 4 m-tiles per group; psum [128,512] per (m, n)
            for g in range(4):
                for n in range(2):
                    pss = []
                    for mi_ in range(4):
                        pss.append(ps_tile())
                    for k in range(MI):
                        for mi_ in range(4):
                            m = g * 4 + mi_
                            nc.tensor.matmul(
                                pss[mi_][:], blks[k][:, ts(m, 128)],
                                u8[:, k, ts(n, 512)],
                                start=(k == 0), stop=(k == MI - 1),
                            )
                    for mi_ in range(4):
                        m = g * 4 + mi_
                        dst = dest_of(m)
                        nc.scalar.activation(
                            out=dst[:, ts(n, 512)], in_=pss[mi_][:],
                            func=func, bias=bias_sb[:, m : m + 1], scale=1.0,
                        )

        # a-phase
        wa_blk = stream_w(wa, MI, INNER, "wa")
        a_tiles = [ay.tile([128, T], bf16, tag="ay", name=f"a_{m}") for m in range(MI)]
        mm_phase(wa_blk, ba_sb, AF.Sigmoid, lambda m: a_tiles[m])

        # b-phase: b -> bu (in place), pass-1 scan, carry assembly
        wb_blk = stream_w(wb, MI, INNER, "wb")
        bu_tiles = [bu.tile([128, T], bf16, tag="bu", name=f"bu_{m}") for m in range(MI)]
        mm_phase(wb_blk, bb_sb, AF.Sigmoid, lambda m: bu_tiles[m])
        for m in range(MI):
            nc.vector.tensor_tensor(
                out=bu_tiles[m][:], in0=bu_tiles[m][:], in1=u8[:, m, :], op=AO.mult
            )
            st1 = stp.tile([128, T], f32, tag="st")
            nc.vector.tensor_tensor_scan(
                out=st1[:], data0=a_tiles[m][:], data1=bu_tiles[m][:],
                initial=0.0, op0=AO.mult, op1=AO.add,
            )
            nc.gpsimd.tensor_copy(out=carry_loc[:, m : m + 1], in_=st1[:, T - 1 : T])
        nc.vector.tensor_scalar(
            out=carry_loc[:], in0=carry_loc[:], scalar1=invsel_sb[:],
            scalar2=None, op0=AO.mult,
        )
        nc.sync.dma_start(out=cc_in[:], in_=carry_loc[:])
        nc.gpsimd.collective_compute(
            "AllReduce", AO.add,
            replica_groups=[[0, 1], [2, 3], [4, 5], [6, 7]],
            ins=[cc_in.opt()], outs=[cc_out.opt()],
        )
        nc.sync.dma_start(out=carry_eff[:], in_=cc_out[:])
        nc.vector.tensor_scalar(
            out=carry_eff[:], in0=carry_eff[:], scalar1=sel_sb[:],
            scalar2=None, op0=AO.mult,
        )

        # c-phase + pass-2 scan + y (y reuses the a-tile slots)
        wc_blk = stream_w(wc, MI, INNER, "wc")
        c_tiles = {}

        def c_dest(m):
            t_ = cp.tile([128, T], bf16, tag="c", name=f"c_{m}")
            c_tiles[m] = t_
            return t_

        mm_phase(wc_blk, bc_sb, AF.Tanh, c_dest)
        y_tiles = []
        for m in range(MI):
            st2 = stp.tile([128, T], f32, tag="st")
            nc.vector.tensor_tensor_scan(
                out=st2[:], data0=a_tiles[m][:], data1=bu_tiles[m][:],
                initial=carry_eff[:, m : m + 1], op0=AO.mult, op1=AO.add,
            )
            t1 = ytmp.tile([128, T], f32, tag="yt")
            nc.vector.tensor_tensor(
                out=t1[:], in0=c_tiles[m][:], in1=st2[:], op=AO.mult
            )
            y = ay.tile([128, T], bf16, tag="ay", name=f"y_{m}")
            nc.vector.tensor_tensor(out=y[:], in0=t1[:], in1=u8[:, m, :], op=AO.add)
            y_tiles.append(y)

        mid.close()  # free u8/bu/cp/stp/ytmp before the Wo-phase pools

        # ---- Wo matmul + transpose back + residual ----------------------
        wo_blk = stream_w(wo, MI, D, "wo")
        with ExitStack() as stO:
            outTp = stO.enter_context(tc.tile_pool(name="outTp", bufs=1))
            otm = stO.enter_context(tc.tile_pool(name="otm", bufs=1))
            xres = stO.enter_context(tc.tile_pool(name="xres", bufs=3))
            ofin = stO.enter_context(tc.tile_pool(name="ofin", bufs=3))
            outT = outTp.tile([128, KD, T], bf16)
            for g in range(2):
                for n in range(2):
                    pss = [ps_tile() for _ in range(4)]
                    for k in range(MI):
                        for di in range(4):
                            d = g * 4 + di
                            nc.tensor.matmul(
                                pss[di][:], wo_blk[k][:, ts(d, 128)],
                                y_tiles[k][:, ts(n, 512)],
                                start=(k == 0), stop=(k == MI - 1),
                            )
                    for di in range(4):
                        d = g * 4 + di
                        nc.scalar.activation(
                            out=outT[:, d, ts(n, 512)], in_=pss[di][:],
                            func=AF.Identity, bias=bo_sb[:, d : d + 1], scale=1.0,
                        )
            out_tm = otm.tile([128, NTOK, D], bf16)
            for d in range(KD):
                nc.sync.dma_start_transpose(
                    out=out_tm[:, :, ts(d, 128)], in_=outT[:, d, :]
                )
            for j in range(NTOK):
                xr = xres.tile([128, D], f32, tag="xr")
                nc.sync.dma_start(out=xr[:], in_=xb[ts(j, 128), :])
                of = ofin.tile([128, D], f32, tag="of")
                nc.vector.tensor_tensor(
                    out=of[:], in0=out_tm[:, j, :], in1=xr[:], op=AO.add
                )
                nc.sync.dma_start(out=out[ts(j, 128), :], in_=of[:])

    _split_waits(nc)
    return nc


_NC_CACHE = {}


def kernel(**inputs):
    x = np.asarray(inputs["x"], np.float32)
    W_in = np.asarray(inputs["W_in"], np.float32)
    b_in = np.asarray(inputs["b_in"], np.float32)
    conv_w = np.asarray(inputs["conv_w"], np.float32)
    conv_b = np.asarray(inputs["conv_b"], np.float32)
    Wa = np.asarray(inputs["Wa"], np.float32)
    ba = np.asarray(inputs["ba"], np.float32)
    Wb = np.asarray(inputs["Wb"], np.float32)
    bb_ = np.asarray(inputs["bb"], np.float32)
    Wc = np.asarray(inputs["Wc"], np.float32)
    bc = np.asarray(inputs["bc"], np.float32)
    Wo = np.asarray(inputs["Wo"], np.float32)
    bo = np.asarray(inputs["bo"], np.float32)
    gamma = np.asarray(inputs["gamma"], np.float32)
    beta = np.asarray(inputs["beta"], np.float32)

    # fold layernorm affine into W_in / b_in
    W_in_f = (gamma[:, None] * W_in).astype(ml_dtypes.bfloat16)
    b_in_f = b_in + beta @ W_in

    def col_t(v, mi):  # [mi*128] -> [128, mi] (partition-major per tile)
        return np.ascontiguousarray(v.reshape(mi, 128).T).astype(np.float32)

    wa_b = Wa.astype(ml_dtypes.bfloat16)
    wb_b = Wb.astype(ml_dtypes.bfloat16)
    wc_b = Wc.astype(ml_dtypes.bfloat16)
    wo_b = Wo.astype(ml_dtypes.bfloat16)
    cw = conv_w[:, 0, :]  # [INNER, 3]
    cw_t = np.ascontiguousarray(
        cw.reshape(MI, 128, 3).transpose(1, 0, 2)
    ).astype(np.float32)  # [128, MI, 3]

    shared = {
        "w_in": W_in_f,
        "wa": wa_b, "wb": wb_b, "wc": wc_b, "wo": wo_b,
        "b_in_t": col_t(b_in_f, 2 * MI),
        "ba_t": col_t(ba, MI), "bb_t": col_t(bb_, MI), "bc_t": col_t(bc, MI),
        "bo_t": col_t(bo, KD), "cw_t": cw_t, "cb_t": col_t(conv_b, MI),
    }

    # host-side halo: projected-channels pre-activation for the 2 tokens
    # before each chunk (zeros at sequence start), in [128, MI, 2] layout
    def halo_of(bi, h):
        if h == 0:
            return np.zeros((128, MI, 2), ml_dtypes.bfloat16)
        xh = x[bi, T - 2 : T, :]  # tokens 1022,1023
        mu = xh.mean(-1, keepdims=True)
        var = ((xh - mu) ** 2).mean(-1, keepdims=True)
        xn = (xh - mu) / np.sqrt(var + EPS)
        pr = (xn * gamma + beta) @ W_in[:, :INNER] + b_in[:INNER]  # [2, INNER]
        return np.ascontiguousarray(
            pr.reshape(2, MI, 128).transpose(2, 1, 0)
        ).astype(ml_dtypes.bfloat16)

    in_maps = []
    for c in range(8):
        bi, h = c // 2, c % 2
        m = dict(shared)
        m["xb"] = np.ascontiguousarray(x[bi, h * T : (h + 1) * T, :])
        m["halo"] = halo_of(bi, h)
        m["sel"] = np.full((1, 1), float(h), np.float32)
        m["invsel"] = np.full((1, 1), float(1 - h), np.float32)
        in_maps.append(m)

    key = "nc"
    if key not in _NC_CACHE:
        _NC_CACHE[key] = build_nc()
    nc = _NC_CACHE[key]

    res = run_bass_kernel_spmd(nc, in_maps, list(range(8)))
    outp = np.empty((B, S, D), np.float32)
    for c in range(8):
        bi, h = c // 2, c % 2
        outp[bi, h * T : (h + 1) * T, :] = res.results[c]["out"]
    return outp


# revision 11
# speedup vs baseline: 7122.1111x; 7122.1111x over previous
"""Trainium2 Bass kernel for CausalGatedSSMBlock.

Sharding: batch(4) x time(2) across 8 cores. Core c handles batch c//2,
token half c%2 (1024 tokens). Each core computes the full block for its
chunk; the only cross-core dependency is the recurrence carry at the
half boundary, exchanged as a [2048] f32 AllReduce within each pair.

On-chip layout is channel-major ([ch, t], "transposed") so the three
inner matmuls and the DVE tensor_tensor_scan all operate natively; the
flips between token-major and channel-major use DMA transpose (bf16).
"""

import os
import sys

sys.path.insert(0, "/opt/trn_rl_repo")
os.environ.setdefault("MYCRO_LOCAL_CACHE", "1")

import numpy as np
import ml_dtypes

import concourse.bass as bass
import concourse.mybir as mybir
import concourse.tile as tile_mod
from concourse.tile import TileContext
from concourse.bass import ts
from concourse.bass_utils import run_bass_kernel_spmd

bf16 = mybir.dt.bfloat16
f32 = mybir.dt.float32
AO = mybir.AluOpType
AF = mybir.ActivationFunctionType

B, S, D, INNER = 4, 2048, 1024, 2048
T = 1024          # tokens per core
NTOK = T // 128   # 8 token tiles
KD = D // 128     # 8 k-tiles of d_model
MI = INNER // 128  # 16 channel tiles of inner dim
EPS = 1e-5

# ---------------------------------------------------------------------------
# This walrus build rejects >1 sem wait on several instruction types
# ("Too many sync wait commands" in setupSyncWait). Keep at most one wait
# per instruction: Tile's kernel-tail drain is rebuilt from single-wait
# drains, and a post-pass hoists overflow waits onto nop carriers that run
# just before the owning instruction on the same engine.
_MAXW = 1
_ctr = [0]


def _patched_drain_and_barrier(self, tick_clock, wait_clock):
    drain_inst = self.nc.sync.drain()
    ins = drain_inst.ins
    wait_clock.add_sem_waits(ins, tile_mod.ScopedClock({None: tick_clock.global_clock}))
    waits = list(ins.sync_info.on_wait)
    if len(waits) > _MAXW:
        ins.sync_info.on_wait = waits[:_MAXW]
        for i in range(_MAXW, len(waits), _MAXW):
            extra = self.nc.sync.drain()
            wait_clock.add_sem_waits(
                extra.ins, tile_mod.ScopedClock({None: tick_clock.global_clock})
            )
            extra.ins.sync_info.on_wait = waits[i : i + _MAXW]
    self.nc.all_engine_barrier()
    popped = self.nc._tile_sem_poison_stack.pop()
    assert popped is self._sem_poison
    self.nc.clear_and_free_semaphores(list(self.sems.allocated().values()))
    self.nc.all_engine_barrier()


TileContext._drain_and_barrier = _patched_drain_and_barrier


def _split_waits(nc, maxw=_MAXW):
    for f in nc.m.functions:
        for bb in f.blocks:
            new = []
            for inst in bb.instructions:
                si = inst.sync_info
                if si is not None:
                    waits = list(si.on_wait)
                    if len(waits) > maxw:
                        keep = waits[-maxw:]
                        extra = waits[:-maxw]
                        for i in range(0, len(extra), maxw):
                            _ctr[0] += 1
                            new.append(
                                mybir.InstNoOp(
                                    name=f"wsplit_{_ctr[0]}",
                                    sync_info=mybir.SyncInfo(
                                        on_wait=extra[i : i + maxw], on_update=[]
                                    ),
                                    bass_nofuse=True,
                                    engine=inst.engine,
                                )
                            )
                        si.on_wait = keep
                new.append(inst)
            bb.instructions = new


# ---------------------------------------------------------------------------


def build_nc():
    nc = bass.Bass()

    xb = nc.declare_dram_parameter("xb", [T, D], f32, isOutput=False)
    halo = nc.declare_dram_parameter("halo", [128, MI, 2], bf16, isOutput=False)
    w_in = nc.declare_dram_parameter("w_in", [D, 2 * INNER], bf16, isOutput=False)
    wa = nc.declare_dram_parameter("wa", [INNER, INNER], bf16, isOutput=False)
    wb = nc.declare_dram_parameter("wb", [INNER, INNER], bf16, isOutput=False)
    wc = nc.declare_dram_parameter("wc", [INNER, INNER], bf16, isOutput=False)
    wo = nc.declare_dram_parameter("wo", [INNER, D], bf16, isOutput=False)
    b_in_t = nc.declare_dram_parameter("b_in_t", [128, 2 * MI], f32, isOutput=False)
    ba_t = nc.declare_dram_parameter("ba_t", [128, MI], f32, isOutput=False)
    bb_t = nc.declare_dram_parameter("bb_t", [128, MI], f32, isOutput=False)
    bc_t = nc.declare_dram_parameter("bc_t", [128, MI], f32, isOutput=False)
    bo_t = nc.declare_dram_parameter("bo_t", [128, KD], f32, isOutput=False)
    cw_t = nc.declare_dram_parameter("cw_t", [128, MI, 3], f32, isOutput=False)
    cb_t = nc.declare_dram_parameter("cb_t", [128, MI], f32, isOutput=False)
    sel_p = nc.declare_dram_parameter("sel", [1, 1], f32, isOutput=False)
    invsel_p = nc.declare_dram_parameter("invsel", [1, 1], f32, isOutput=False)
    out = nc.declare_dram_parameter("out", [T, D], f32, isOutput=True)

    from contextlib import ExitStack

    with TileContext(nc) as tc, ExitStack() as big:
        consts = big.enter_context(tc.tile_pool(name="consts", bufs=1))
        wstream = big.enter_context(tc.tile_pool(name="wstream", bufs=17))
        psum = big.enter_context(tc.tile_pool(name="psum", bufs=8, space="PSUM"))
        _pc = [0]
        def ps_tile():
            _pc[0] += 1
            return psum.tile([128, 512], f32, tag="ps", name=f"ps_{_pc[0]}")
        dram = big.enter_context(tc.tile_pool(name="dram", bufs=1, space="DRAM"))

        # ---- constants --------------------------------------------------
        binp_sb = consts.tile([128, 2 * MI], f32)
        nc.sync.dma_start(out=binp_sb[:], in_=b_in_t[:])
        ba_sb = consts.tile([128, MI], f32)
        nc.sync.dma_start(out=ba_sb[:], in_=ba_t[:])
        bb_sb = consts.tile([128, MI], f32)
        nc.sync.dma_start(out=bb_sb[:], in_=bb_t[:])
        bc_sb = consts.tile([128, MI], f32)
        nc.sync.dma_start(out=bc_sb[:], in_=bc_t[:])
        bo_sb = consts.tile([128, KD], f32)
        nc.sync.dma_start(out=bo_sb[:], in_=bo_t[:])
        cw_sb = consts.tile([128, MI, 3], f32)
        nc.sync.dma_start(out=cw_sb[:], in_=cw_t[:])
        cb_sb = consts.tile([128, MI], f32)
        nc.sync.dma_start(out=cb_sb[:], in_=cb_t[:])
        sel_sb = consts.tile([128, 1], f32)
        nc.sync.dma_start(out=sel_sb[:], in_=sel_p[:].to_broadcast((128, 1)))
        invsel_sb = consts.tile([128, 1], f32)
        nc.sync.dma_start(out=invsel_sb[:], in_=invsel_p[:].to_broadcast((128, 1)))
        eps_sb = consts.tile([128, 1], f32)
        nc.vector.memset(eps_sb[:], EPS)

        carry_loc = consts.tile([128, MI], f32)
        carry_eff = consts.tile([128, MI], f32)
        cc_in = dram.tile([128, MI], f32)
        cc_out = dram.tile([128, MI], f32)

        # ---- weight streaming: uniform [128, 2048] bf16 blocks ----------
        # W_in: 8 k-blocks x 2 m-halves; Wa/Wb/Wc: 16 k-blocks;
        # Wo: 16 k-blocks of [128, 1024] (same tag, smaller tile).
        win_blk = {}
        for k in range(KD):
            for mh in range(2):
                t_ = wstream.tile([128, INNER], bf16, tag="wblk", name=f"win_{k}_{mh}")
                nc.sync.dma_start(
                    out=t_[:], in_=w_in[ts(k, 128), ts(mh, INNER)]
                )
                win_blk[(k, mh)] = t_

        # ay outlives the mid-section (y tiles feed the Wo matmul), so its
        # pool must be entered before u8/xnT per LIFO pool ordering.
        ay = big.enter_context(tc.tile_pool(name="ay", bufs=MI + 1))
        mid = ExitStack()  # u8 + a/b/c-phase transients; closed before Wo
        u8 = mid.enter_context(tc.tile_pool(name="u8", bufs=1)).tile(
            [128, MI, T], bf16
        )

        # ---- stage A: layernorm + transpose -----------------------------
        stAB = ExitStack()  # pools freed after stage B
        xnT_pool = stAB.enter_context(tc.tile_pool(name="xnT", bufs=1))
        xnT = xnT_pool.tile([128, KD, T], bf16)
        with ExitStack() as stA:
            xln = stA.enter_context(tc.tile_pool(name="xln", bufs=3))
            xnp = stA.enter_context(tc.tile_pool(name="xnp", bufs=3))
            stat = stA.enter_context(tc.tile_pool(name="stat", bufs=6))
            for j in range(NTOK):
                xt = xln.tile([128, D], f32)
                nc.sync.dma_start(out=xt[:], in_=xb[ts(j, 128), :])
                stats = stat.tile([128, 2, 6], f32)
                xr = xt[:].rearrange("p (s q) -> p s q", s=2)
                for s_ in range(2):
                    nc.vector.bn_stats(out=stats[:, s_, :], in_=xr[:, s_, :])
                mv = stat.tile([128, 2], f32)
                nc.vector.bn_aggr(out=mv[:], in_=stats[:])
                rstd = stat.tile([128, 1], f32)
                nc.scalar.activation(
                    out=rstd[:], in_=mv[:, 1:2], func=AF.Sqrt,
                    bias=eps_sb[:], scale=1.0,
                )
                nc.vector.reciprocal(out=rstd[:], in_=rstd[:])
                xn = xnp.tile([128, D], bf16)
                nc.vector.tensor_scalar(
                    out=xn[:], in0=xt[:], scalar1=mv[:, 0:1], scalar2=rstd[:],
                    op0=AO.subtract, op1=AO.mult,
                )
                nc.sync.dma_start_transpose(out=xnT[:, :, ts(j, 128)], in_=xn[:])

        # ---- stage B: W_in matmul + conv + gated silu -> u --------------
        with ExitStack() as stB:
            projp = stB.enter_context(tc.tile_pool(name="projp", bufs=3))
            sgp = stB.enter_context(tc.tile_pool(name="sgp", bufs=3))
            cvt = stB.enter_context(tc.tile_pool(name="cvt", bufs=4))
            for m in range(MI):
                proj = projp.tile([128, 2 + T], bf16, tag="proj")
                nc.sync.dma_start(out=proj[:, 0:2], in_=halo[:, m, :])
                for n in range(2):
                    ps = ps_tile()
                    for k in range(KD):
                        nc.tensor.matmul(
                            ps[:], win_blk[(k, 0)][:, ts(m, 128)],
                            xnT[:, k, ts(n, 512)],
                            start=(k == 0), stop=(k == KD - 1),
                        )
                    nc.scalar.activation(
                        out=proj[:, 2 + n * 512 : 2 + (n + 1) * 512], in_=ps[:],
                        func=AF.Identity, bias=binp_sb[:, m : m + 1], scale=1.0,
                    )
                sg = sgp.tile([128, T], bf16, tag="sg")
                for n in range(2):
                    ps = ps_tile()
                    for k in range(KD):
                        nc.tensor.matmul(
                            ps[:], win_blk[(k, 1)][:, ts(m, 128)],
                            xnT[:, k, ts(n, 512)],
                            start=(k == 0), stop=(k == KD - 1),
                        )
                    nc.scalar.activation(
                        out=sg[:, ts(n, 512)], in_=ps[:],
                        func=AF.Sigmoid, bias=binp_sb[:, MI + m : MI + m + 1],
                        scale=1.0,
                    )
                # causal depthwise conv (3 taps) + bias + silu
                c1 = cvt.tile([128, T], f32, tag="cv")
                nc.vector.tensor_scalar(
                    out=c1[:], in0=proj[:, 0:T], scalar1=cw_sb[:, m, 0:1],
                    scalar2=None, op0=AO.mult,
                )
                c2 = cvt.tile([128, T], f32, tag="cv")
                nc.vector.scalar_tensor_tensor(
                    out=c2[:], in0=proj[:, 1 : 1 + T], scalar=cw_sb[:, m, 1:2],
                    in1=c1[:], op0=AO.mult, op1=AO.add,
                )
                c3 = cvt.tile([128, T], f32, tag="cv")
                nc.vector.scalar_tensor_tensor(
                    out=c3[:], in0=proj[:, 2 : 2 + T], scalar=cw_sb[:, m, 2:3],
                    in1=c2[:], op0=AO.mult, op1=AO.add,
                )
                sil = cvt.tile([128, T], bf16, tag="sil")
                nc.scalar.activation(
                    out=sil[:], in_=c3[:], func=AF.Silu,
                    bias=cb_sb[:, m : m + 1], scale=1.0,
                )
                nc.vector.tensor_tensor(
                    out=u8[:, m, :], in0=sil[:], in1=sg[:], op=AO.mult
                )
        stAB.close()  # free xnT before the a/b/c phases

        # ---- stage a/b/c matmuls + scans --------------------------------
        bu = mid.enter_context(tc.tile_pool(name="bu", bufs=MI))
        cp = mid.enter_context(tc.tile_pool(name="cp", bufs=3))
        stp = mid.enter_context(tc.tile_pool(name="stp", bufs=2))
        ytmp = mid.enter_context(tc.tile_pool(name="ytmp", bufs=2))

        def stream_w(param, nk, width, pfx):
            blks = []
            for k in range(nk):
                t_ = wstream.tile([128, width], bf16, tag="wblk", name=f"{pfx}_{k}")
                nc.sync.dma_start(out=t_[:], in_=param[ts(k, 128), :])
                blks.append(t_)
            return blks

        def mm_phase(blks, bias_sb, func, dest_of):
            # 4 m-tiles per group; psum [128,512] per (m, n)
            for g in range(4):
                for n in range(2):
                    pss = []
                    for mi_ in range(4):
                        pss.append(ps_tile())
                    for k in range(MI):
                        for mi_ in range(4):
                            m = g * 4 + mi_
                            nc.tensor.matmul(
                                pss[mi_][:], blks[k][:, ts(m, 128)],
                                u8[:, k, ts(n, 512)],
                                start=(k == 0), stop=(k == MI - 1),
                            )
                    for mi_ in range(4):
                        m = g * 4 + mi_
                        dst = dest_of(m)
                        nc.scalar.activation(
                            out=dst[:, ts(n, 512)], in_=pss[mi_][:],
                            func=func, bias=bias_sb[:, m : m + 1], scale=1.0,
                        )

        # a-phase
        wa_blk = stream_w(wa, MI, INNER, "wa")
        a_tiles = [ay.tile([128, T], bf16, tag="ay", name=f"a_{m}") for m in range(MI)]
        mm_phase(wa_blk, ba_sb, AF.Sigmoid, lambda m: a_tiles[m])

        # b-phase: b -> bu (in place), pass-1 scan, carry assembly
        wb_blk = stream_w(wb, MI, INNER, "wb")
        bu_tiles = [bu.tile([128, T], bf16, tag="bu", name=f"bu_{m}") for m in range(MI)]
        mm_phase(wb_blk, bb_sb, AF.Sigmoid, lambda m: bu_tiles[m])
        for m in range(MI):
            nc.vector.tensor_tensor(
                out=bu_tiles[m][:], in0=bu_tiles[m][:], in1=u8[:, m, :], op=AO.mult
            )
            st1 = stp.tile([128, T], f32, tag="st")
            nc.vector.tensor_tensor_scan(
                out=st1[:], data0=a_tiles[m][:], data1=bu_tiles[m][:],
                initial=0.0, op0=AO.mult, op1=AO.add,
            )
            nc.gpsimd.tensor_copy(out=carry_loc[:, m : m + 1], in_=st1[:, T - 1 : T])
        nc.vector.tensor_scalar(
            out=carry_loc[:], in0=carry_loc[:], scalar1=invsel_sb[:],
            scalar2=None, op0=AO.mult,
        )
        nc.sync.dma_start(out=cc_in[:], in_=carry_loc[:])
        nc.gpsimd.collective_compute(
            "AllReduce", AO.add,
            replica_groups=[[0, 1], [2, 3], [4, 5], [6, 7]],
            ins=[cc_in.opt()], outs=[cc_out.opt()],
        )
        nc.sync.dma_start(out=carry_eff[:], in_=cc_out[:])
        nc.vector.tensor_scalar(
            out=carry_eff[:], in0=carry_eff[:], scalar1=sel_sb[:],
            scalar2=None, op0=AO.mult,
        )

        # c-phase + pass-2 scan + y (y reuses the a-tile slots)
        wc_blk = stream_w(wc, MI, INNER, "wc")
        c_tiles = {}

        def c_dest(m):
            t_ = cp.tile([128, T], bf16, tag="c", name=f"c_{m}")
            c_tiles[m] = t_
            return t_

        mm_phase(wc_blk, bc_sb, AF.Tanh, c_dest)
        y_tiles = []
        for m in range(MI):
            st2 = stp.tile([128, T], f32, tag="st")
            nc.vector.tensor_tensor_scan(
                out=st2[:], data0=a_tiles[m][:], data1=bu_tiles[m][:],
                initial=carry_eff[:, m : m + 1], op0=AO.mult, op1=AO.add,
            )
            t1 = ytmp.tile([128, T], f32, tag="yt")
            nc.vector.tensor_tensor(
                out=t1[:], in0=c_tiles[m][:], in1=st2[:], op=AO.mult
            )
            y = ay.tile([128, T], bf16, tag="ay", name=f"y_{m}")
            nc.vector.tensor_tensor(out=y[:], in0=t1[:], in1=u8[:, m, :], op=AO.add)
            y_tiles.append(y)

        mid.close()  # free u8/bu/cp/stp/ytmp before the Wo-phase pools

        # ---- Wo matmul + transpose back + residual ----------------------
        wo_blk = stream_w(wo, MI, D, "wo")
        with ExitStack() as stO:
            outTp = stO.enter_context(tc.tile_pool(name="outTp", bufs=1))
            otm = stO.enter_context(tc.tile_pool(name="otm", bufs=1))
            xres = stO.enter_context(tc.tile_pool(name="xres", bufs=3))
            ofin = stO.enter_context(tc.tile_pool(name="ofin", bufs=3))
            outT = outTp.tile([128, KD, T], bf16)
            for g in range(2):
                for n in range(2):
                    pss = [ps_tile() for _ in range(4)]
                    for k in range(MI):
                        for di in range(4):
                            d = g * 4 + di
                            nc.tensor.matmul(
                                pss[di][:], wo_blk[k][:, ts(d, 128)],
                                y_tiles[k][:, ts(n, 512)],
                                start=(k == 0), stop=(k == MI - 1),
                            )
                    for di in range(4):
                        d = g * 4 + di
                        nc.scalar.activation(
                            out=outT[:, d, ts(n, 512)], in_=pss[di][:],
                            func=AF.Identity, bias=bo_sb[:, d : d + 1], scale=1.0,
                        )
            out_tm = otm.tile([128, NTOK, D], bf16)
            for d in range(KD):
                nc.sync.dma_start_transpose(
                    out=out_tm[:, :, ts(d, 128)], in_=outT[:, d, :]
                )
            for j in range(NTOK):
                xr = xres.tile([128, D], f32, tag="xr")
                nc.sync.dma_start(out=xr[:], in_=xb[ts(j, 128), :])
                of = ofin.tile([128, D], f32, tag="of")
                nc.vector.tensor_tensor(
                    out=of[:], in0=out_tm[:, j, :], in1=xr[:], op=AO.add
                )
                nc.sync.dma_start(out=out[ts(j, 128), :], in_=of[:])

    _split_waits(nc)
    return nc


_NC_CACHE = {}
_LAST_EXEC_NS = None


def kernel(**inputs):
    x = np.asarray(inputs["x"], np.float32)
    W_in = np.asarray(inputs["W_in"], np.float32)
    b_in = np.asarray(inputs["b_in"], np.float32)
    conv_w = np.asarray(inputs["conv_w"], np.float32)
    conv_b = np.asarray(inputs["conv_b"], np.float32)
    Wa = np.asarray(inputs["Wa"], np.float32)
    ba = np.asarray(inputs["ba"], np.float32)
    Wb = np.asarray(inputs["Wb"], np.float32)
    bb_ = np.asarray(inputs["bb"], np.float32)
    Wc = np.asarray(inputs["Wc"], np.float32)
    bc = np.asarray(inputs["bc"], np.float32)
    Wo = np.asarray(inputs["Wo"], np.float32)
    bo = np.asarray(inputs["bo"], np.float32)
    gamma = np.asarray(inputs["gamma"], np.float32)
    beta = np.asarray(inputs["beta"], np.float32)

    # fold layernorm affine into W_in / b_in
    W_in_f = (gamma[:, None] * W_in).astype(ml_dtypes.bfloat16)
    b_in_f = b_in + beta @ W_in

    def col_t(v, mi):  # [mi*128] -> [128, mi] (partition-major per tile)
        return np.ascontiguousarray(v.reshape(mi, 128).T).astype(np.float32)

    wa_b = Wa.astype(ml_dtypes.bfloat16)
    wb_b = Wb.astype(ml_dtypes.bfloat16)
    wc_b = Wc.astype(ml_dtypes.bfloat16)
    wo_b = Wo.astype(ml_dtypes.bfloat16)
    cw = conv_w[:, 0, :]  # [INNER, 3]
    cw_t = np.ascontiguousarray(
        cw.reshape(MI, 128, 3).transpose(1, 0, 2)
    ).astype(np.float32)  # [128, MI, 3]

    shared = {
        "w_in": W_in_f,
        "wa": wa_b, "wb": wb_b, "wc": wc_b, "wo": wo_b,
        "b_in_t": col_t(b_in_f, 2 * MI),
        "ba_t": col_t(ba, MI), "bb_t": col_t(bb_, MI), "bc_t": col_t(bc, MI),
        "bo_t": col_t(bo, KD), "cw_t": cw_t, "cb_t": col_t(conv_b, MI),
    }

    # host-side halo: projected-channels pre-activation for the 2 tokens
    # before each chunk (zeros at sequence start), in [128, MI, 2] layout
    def halo_of(bi, h):
        if h == 0:
            return np.zeros((128, MI, 2), ml_dtypes.bfloat16)
        xh = x[bi, T - 2 : T, :]  # tokens 1022,1023
        mu = xh.mean(-1, keepdims=True)
        var = ((xh - mu) ** 2).mean(-1, keepdims=True)
        xn = (xh - mu) / np.sqrt(var + EPS)
        pr = (xn * gamma + beta) @ W_in[:, :INNER] + b_in[:INNER]  # [2, INNER]
        return np.ascontiguousarray(
            pr.reshape(2, MI, 128).transpose(2, 1, 0)
        ).astype(ml_dtypes.bfloat16)

    in_maps = []
    for c in range(8):
        bi, h = c // 2, c % 2
        m = dict(shared)
        m["xb"] = np.ascontiguousarray(x[bi, h * T : (h + 1) * T, :])
        m["halo"] = halo_of(bi, h)
        m["sel"] = np.full((1, 1), float(h), np.float32)
        m["invsel"] = np.full((1, 1), float(1 - h), np.float32)
        in_maps.append(m)

    key = "nc"
    if key not in _NC_CACHE:
        _NC_CACHE[key] = build_nc()
    nc = _NC_CACHE[key]

    trace = os.environ.get("KERNEL_TRACE", "0") == "1"
    if trace:
        try:
            res = run_bass_kernel_spmd(nc, in_maps, list(range(8)), trace=True)
        except Exception as e:
            print(f"trace run failed ({e!r}); rerunning without trace")
            res = run_bass_kernel_spmd(nc, in_maps, list(range(8)))
    else:
        res = run_bass_kernel_spmd(nc, in_maps, list(range(8)))
    global _LAST_EXEC_NS
    _LAST_EXEC_NS = getattr(res, "exec_time_ns", None)
    outp = np.empty((B, S, D), np.float32)
    for c in range(8):
        bi, h = c // 2, c % 2
        outp[bi, h * T : (h + 1) * T, :] = res.results[c]["out"]
    return outp
